# revision 1
# baseline (speedup 1.0000x reference)
"""Deformable Conv2d (K=3, stride 1, pad 1, dil 1) on 8 TRN2 NeuronCores.

Sharding: data-parallel over (batch=4) x (H halves=2) -> 8 cores.
Each core computes out[b, :, h0:h0+64, :] for its (b, h0).

Per-core device pipeline:
  1. offset conv (18ch) via PE matmuls over a 1px-zero-padded image.
  2. PE-transpose offsets to point-major layout [128pts, 18].
  3. DVE coord math: ys/xs, floor (magic-number), frac, clamp, int32
     gather indices into a 2px-zero-padded channels-last image in DRAM.
  4. Pool-engine indirect DMA gather: per (point, tap, y-row) one 512B run
     (2 adjacent pixels x 64 channels) -> [128pts, taps*2rows*128].
  5. DVE bilinear lerp (x then y) -> sampled S [128pts, 9taps*64ch].
  6. PE transpose S back to channel-major, main conv matmuls (K=576 as
     4x128+64 accumulation), ACT bias add, DMA out.
"""

import sys
for p in ("/opt/trn_rl_repo",):
    if p not in sys.path:
        sys.path.insert(0, p)

import numpy as np

import concourse.bacc as bacc
import concourse.mybir as mybir
import concourse.tile as tile
import concourse.bass as bass
from concourse.bass import IndirectOffsetOnAxis
from concourse.bass_utils import run_bass_kernel_spmd

F32 = mybir.dt.float32
I32 = mybir.dt.int32
AL = mybir.AluOpType
AF = mybir.ActivationFunctionType

B, C, H, W = 4, 64, 128, 128
K, KK = 3, 9
O = 64                      # output channels
OC = 2 * KK                 # offset channels (18)
HL = H // 2                 # local rows per core (64)
NPT = HL * W                # local points per core (8192)
NG = NPT // 128             # point groups of 128 (=64); group g == local row g
W2 = W + 2                  # 1px-padded width for offset conv (130)
H2 = HL + 2                 # 1px-padded local rows (66)
W4 = W + 4                  # 2px-padded width for gather image (132)
H4 = H + 4                  # 2px-padded height (full image!) (132)
MAGIC = float(3 * 2 ** 22)   # 1.5*2^23: ulp stays 1.0 for inputs in [-2^22, 2^22]
GCH = 2                     # point-groups per gather instruction
GBLK = OC * 128             # gathered elems per point per group-block (2304)


def build_program(dbg=False, skip_gather=False, skip_lerp=False,
                  skip_mm=False, skip_off=False):
    nc = bacc.Bacc("TRN2", target_bir_lowering=False, debug=False)

    xp = nc.dram_tensor("xp", [C, H2 * W2], F32, kind="ExternalInput")
    xcl = nc.dram_tensor("xcl", [H4 * W4, 4 * C], F32, kind="ExternalInput")
    wofft = nc.dram_tensor("wofft", [C, KK * OC], F32, kind="ExternalInput")
    woffb = nc.dram_tensor("woffb", [OC, 1], F32, kind="ExternalInput")
    wmain = nc.dram_tensor("wmain", [128, 5 * O], F32, kind="ExternalInput")
    wb = nc.dram_tensor("wb", [O, 1], F32, kind="ExternalInput")
    basey = nc.dram_tensor("basey", [128, NG * KK], F32, kind="ExternalInput")
    basex = nc.dram_tensor("basex", [128, NG * KK], F32, kind="ExternalInput")
    ident = nc.dram_tensor("ident", [128, 128], F32, kind="ExternalInput")
    out = nc.dram_tensor("out", [O, NPT], F32, kind="ExternalOutput")
    if dbg:
        d_off = nc.dram_tensor("d_off", [OC, NPT], F32, kind="ExternalOutput")
        d_fx = nc.dram_tensor("d_fx", [128, NG * KK], F32, kind="ExternalOutput")
        d_fy = nc.dram_tensor("d_fy", [128, NG * KK], F32, kind="ExternalOutput")
        d_idx = nc.dram_tensor("d_idx", [128, NG * OC], I32, kind="ExternalOutput")
        d_s = nc.dram_tensor("d_s", [128, NG * KK * C], F32, kind="ExternalOutput")
        d_g = nc.dram_tensor("d_g", [128, NG * GBLK], F32, kind="ExternalOutput")

    with tile.TileContext(nc) as tc:
        with (
            tc.tile_pool(name="cst", bufs=1) as cst,
            tc.tile_pool(name="keep", bufs=1) as keep,
            tc.tile_pool(name="psA", bufs=3, space="PSUM") as psA,
            tc.tile_pool(name="psO", bufs=2, space="PSUM") as psO,
        ):
            # ---- load constants / weights ----
            ident_t = cst.tile([128, 128], F32, tag="ident")
            nc.sync.dma_start(out=ident_t[:], in_=ident[:])
            wofft_t = cst.tile([C, KK * OC], F32, tag="wofft")
            nc.sync.dma_start(out=wofft_t[:], in_=wofft[:])
            woffb_t = cst.tile([OC, 1], F32, tag="woffb")
            nc.sync.dma_start(out=woffb_t[:], in_=woffb[:])
            wmain_t = cst.tile([128, 5 * O], F32, tag="wmain")
            nc.sync.dma_start(out=wmain_t[:], in_=wmain[:])
            wb_t = cst.tile([O, 1], F32, tag="wb")
            nc.sync.dma_start(out=wb_t[:], in_=wb[:])
            basey_t = cst.tile([128, NG * KK], F32, tag="basey")
            nc.sync.dma_start(out=basey_t[:], in_=basey[:])
            basex_t = cst.tile([128, NG * KK], F32, tag="basex")
            nc.sync.dma_start(out=basex_t[:], in_=basex[:])

            fy = keep.tile([128, NG * KK], F32, tag="fy")
            fx = keep.tile([128, NG * KK], F32, tag="fx")
            idx = keep.tile([128, NG * OC], I32, tag="idx")

            with (
                tc.tile_pool(name="early", bufs=1) as early,
                tc.tile_pool(name="tmp", bufs=1) as tmp,
            ):
                xp_t = early.tile([C, H2 * W2], F32, tag="xp")
                nc.sync.dma_start(out=xp_t[:], in_=xp[:])
                xp3 = xp_t[:].rearrange("c (h w) -> c h w", h=H2)

                # ---- offset conv: OFF[18, NPT] ----
                off_t = early.tile([OC, NPT], F32, tag="off")
                RPC = 4                       # rows per psum chunk (N=512)
                for r0 in ([] if skip_off else range(0, HL, RPC)):
                    ps = psA.tile([OC, RPC * W], F32, tag="psA")
                    for kk in range(KK):
                        ki, kj = kk // K, kk % K
                        rhs = xp3[:, r0 + ki:r0 + ki + RPC, kj:kj + W]
                        nc.tensor.matmul(
                            out=ps[:], lhsT=wofft_t[:, kk * OC:(kk + 1) * OC],
                            rhs=rhs, start=(kk == 0), stop=(kk == KK - 1))
                    nc.scalar.activation(
                        out=off_t[:, r0 * W:(r0 + RPC) * W], in_=ps[:],
                        func=AF.Identity, bias=woffb_t[:, 0:1], scale=1.0)

                # ---- transpose offsets to point-major: OFF_T[128, NG*18] ----
                offT = tmp.tile([128, NG * OC], F32, tag="offT")
                for g in range(NG):
                    ps = psA.tile([128, OC], F32, tag="psA")
                    nc.tensor.transpose(
                        out=ps[:], in_=off_t[:, g * 128:(g + 1) * 128],
                        identity=ident_t[:OC, :OC])
                    nc.scalar.copy(out=offT[:, g * OC:(g + 1) * OC], in_=ps[:])

                # ---- coordinate math (all wide [128, NG*KK] ops) ----
                NW = NG * KK
                o4 = offT[:].rearrange("p (g k t) -> p g k t", g=NG, k=KK)
                dy = o4[:, :, :, 0]
                dx = o4[:, :, :, 1]

                ys = tmp.tile([128, NW], F32, tag="ys")
                xs = tmp.tile([128, NW], F32, tag="xs")
                rr = tmp.tile([128, NW], F32, tag="rr")
                mm = tmp.tile([128, NW], F32, tag="mm")
                y0 = tmp.tile([128, NW], F32, tag="y0")
                x0 = tmp.tile([128, NW], F32, tag="x0")
                ti = tmp.tile([128, NW], F32, tag="ti")

                ys3 = ys[:].rearrange("p (g k) -> p g k", g=NG)
                xs3 = xs[:].rearrange("p (g k) -> p g k", g=NG)
                by3 = basey_t[:].rearrange("p (g k) -> p g k", g=NG)
                bx3 = basex_t[:].rearrange("p (g k) -> p g k", g=NG)
                nc.vector.tensor_tensor(out=ys3, in0=dy, in1=by3, op=AL.add)
                nc.vector.tensor_tensor(out=xs3, in0=dx, in1=bx3, op=AL.add)

                def floorv(src, dst, frac):
                    # magic-number round-to-nearest, then fix round-ups
                    nc.vector.tensor_scalar(
                        out=rr[:], in0=src[:], scalar1=MAGIC, scalar2=MAGIC,
                        op0=AL.add, op1=AL.subtract)
                    nc.vector.tensor_tensor(out=mm[:], in0=rr[:], in1=src[:],
                                            op=AL.is_gt)
                    nc.vector.tensor_tensor(out=dst[:], in0=rr[:], in1=mm[:],
                                            op=AL.subtract)
                    nc.vector.tensor_tensor(out=frac[:], in0=src[:], in1=dst[:],
                                            op=AL.subtract)

                floorv(ys, y0, fy)
                floorv(xs, x0, fx)
                # clamp (reuse rr/mm as clamped outputs)
                nc.vector.tensor_scalar(out=rr[:], in0=y0[:], scalar1=-2.0,
                                        scalar2=float(H), op0=AL.max, op1=AL.min)
                nc.vector.tensor_scalar(out=mm[:], in0=x0[:], scalar1=-2.0,
                                        scalar2=float(W), op0=AL.max, op1=AL.min)
                # ti = y0c*W4 + x0c ; idx0 = int(ti + 2*W4+2) ; idx1 = idx0 + W4
                nc.vector.scalar_tensor_tensor(
                    out=ti[:], in0=rr[:], scalar=float(W4), in1=mm[:],
                    op0=AL.mult, op1=AL.add)
                i4 = idx[:].rearrange("p (g k t) -> p g k t", g=NG, k=KK)
                ti3 = ti[:].rearrange("p (g k) -> p g k", g=NG)
                nc.vector.tensor_scalar(
                    out=i4[:, :, :, 0], in0=ti3, scalar1=float(2 * W4 + 2),
                    scalar2=None, op0=AL.add)
                nc.vector.tensor_scalar(
                    out=i4[:, :, :, 1], in0=i4[:, :, :, 0], scalar1=W4,
                    scalar2=None, op0=AL.add)
                if dbg:
                    nc.sync.dma_start(out=d_off[:], in_=off_t[:])
                    nc.sync.dma_start(out=d_fx[:], in_=fx[:])
                    nc.sync.dma_start(out=d_fy[:], in_=fy[:])
                    nc.sync.dma_start(out=d_idx[:], in_=idx[:])

            # ---- main loop: gather -> lerp -> transpose -> matmul ----
            with (
                tc.tile_pool(name="gat", bufs=2) as gat,
                tc.tile_pool(name="lrp", bufs=2) as lrp,
                tc.tile_pool(name="outp", bufs=1) as outp,
            ):
                out_sb = outp.tile([O, NPT], F32, tag="osb")
                if skip_mm:
                    nc.vector.memset(out_sb[:], 0.0)
                for c0 in range(0, NG, GCH):
                    gt = gat.tile([128, GCH * GBLK], F32, tag="G")
                    if skip_gather and not skip_lerp:
                        nc.vector.memset(gt[:], 0.0)
                    # one [P,1]-offset indirect DMA per (group, tap, y-row):
                    # each partition reads 128 contiguous f32 (2 adjacent px
                    # x 64ch) from its own offset.  HW semantics: per
                    # partition, one offset + contiguous continuation.
                    for gs in ([] if skip_gather else range(GCH)):
                        for kk in range(KK):
                            col = ((c0 + gs) * OC + kk * 2)
                            nc.gpsimd.indirect_dma_start(
                                out=gt[:, (gs * KK + kk) * 256:
                                       (gs * KK + kk + 1) * 256],
                                out_offset=None, in_=xcl[:],
                                in_offset=IndirectOffsetOnAxis(
                                    ap=idx[:, col:col + 1], axis=0))
                    for gs in range(GCH):
                        g = c0 + gs
                        g5 = gt[:, gs * GBLK:(gs + 1) * GBLK].rearrange(
                            "p (k r q c) -> p k r q c", k=KK, r=2, q=2)
                        v00 = g5[:, :, 0, 0, :]
                        v01 = g5[:, :, 0, 1, :]
                        v10 = g5[:, :, 1, 0, :]
                        v11 = g5[:, :, 1, 1, :]
                        fxb = fx[:, g * KK:(g + 1) * KK].unsqueeze(2) \
                            .to_broadcast([128, KK, C])
                        fyb = fy[:, g * KK:(g + 1) * KK].unsqueeze(2) \
                            .to_broadcast([128, KK, C])

                        d_ = lrp.tile([128, KK * C], F32, tag="d")
                        m_ = lrp.tile([128, KK * C], F32, tag="m")
                        l0 = lrp.tile([128, KK * C], F32, tag="l0")
                        l1 = lrp.tile([128, KK * C], F32, tag="l1")
                        s_ = lrp.tile([128, KK * C], F32, tag="s")
                        if skip_lerp and not skip_mm:
                            nc.vector.memset(s_[:], 0.0)
                        d3 = d_[:].rearrange("p (k c) -> p k c", k=KK)
                        m3 = m_[:].rearrange("p (k c) -> p k c", k=KK)
                        l03 = l0[:].rearrange("p (k c) -> p k c", k=KK)
                        l13 = l1[:].rearrange("p (k c) -> p k c", k=KK)
                        s3 = s_[:].rearrange("p (k c) -> p k c", k=KK)

                        if skip_lerp:
                            pass
                        else:
                            nc.vector.tensor_tensor(out=d3, in0=v01, in1=v00, op=AL.subtract)
                        if not skip_lerp:
                            nc.vector.tensor_tensor(out=m3, in0=d3, in1=fxb, op=AL.mult)
                            nc.vector.tensor_tensor(out=l03, in0=m3, in1=v00, op=AL.add)
                            nc.vector.tensor_tensor(out=d3, in0=v11, in1=v10, op=AL.subtract)
                            nc.vector.tensor_tensor(out=m3, in0=d3, in1=fxb, op=AL.mult)
                            nc.vector.tensor_tensor(out=l13, in0=m3, in1=v10, op=AL.add)
                            nc.vector.tensor_tensor(out=d3, in0=l13, in1=l03, op=AL.subtract)
                            nc.vector.tensor_tensor(out=m3, in0=d3, in1=fyb, op=AL.mult)
                            nc.vector.tensor_tensor(out=s3, in0=m3, in1=l03, op=AL.add)

                        if dbg:
                            nc.sync.dma_start(
                                out=d_s[:, g * KK * C:(g + 1) * KK * C], in_=s_[:])
                            nc.sync.dma_start(
                                out=d_g[:, g * GBLK:(g + 1) * GBLK],
                                in_=gt[:, gs * GBLK:(gs + 1) * GBLK])

                        # transpose S to channel-major tap-pair blocks
                        st = lrp.tile([128, 640], F32, tag="st")
                        for j in ([] if skip_mm else range(4)):
                            ps = psA.tile([128, 128], F32, tag="psA")
                            nc.tensor.transpose(
                                out=ps[:], in_=s_[:, j * 128:(j + 1) * 128],
                                identity=ident_t[:])
                            nc.scalar.copy(out=st[:, j * 128:(j + 1) * 128], in_=ps[:])
                        if not skip_mm:
                            ps = psA.tile([64, 128], F32, tag="psA")
                            nc.tensor.transpose(
                                out=ps[:], in_=s_[:, 512:576], identity=ident_t[:])
                            nc.scalar.copy(out=st[:64, 512:640], in_=ps[:])

                            po = psO.tile([O, 128], F32, tag="psO")
                            for j in range(4):
                                nc.tensor.matmul(
                                    out=po[:], lhsT=wmain_t[:, j * O:(j + 1) * O],
                                    rhs=st[:, j * 128:(j + 1) * 128],
                                    start=(j == 0), stop=False)
                            nc.tensor.matmul(
                                out=po[:], lhsT=wmain_t[:64, 4 * O:5 * O],
                                rhs=st[:64, 512:640], start=False, stop=True)
                            nc.scalar.activation(
                                out=out_sb[:, g * 128:(g + 1) * 128], in_=po[:],
                                func=AF.Identity, bias=wb_t[:, 0:1], scale=1.0)

            nc.sync.dma_start(out=out[:], in_=out_sb[:])

    nc.compile()
    return nc


_NC_CACHE = None


def _get_nc():
    global _NC_CACHE
    if _NC_CACHE is None:
        _NC_CACHE = build_program()
    return _NC_CACHE


def make_core_inputs(x, weight, bias, offset_w, offset_b):
    """Host-side prep: returns list of 8 in_maps (core i = batch i//2, half i%2)."""
    x = np.asarray(x, np.float32)
    weight = np.asarray(weight, np.float32)
    bias = np.asarray(bias, np.float32)
    offset_w = np.asarray(offset_w, np.float32)
    offset_b = np.asarray(offset_b, np.float32)

    xp_full = np.pad(x, ((0, 0), (0, 0), (1, 1), (1, 1)))
    xpad = np.pad(x, ((0, 0), (0, 0), (2, 2), (2, 3)))  # extra right/bottom col for i+1/i+133
    xpad = np.pad(xpad, ((0, 0), (0, 0), (0, 1), (0, 0)))
    xcl0 = xpad.transpose(0, 2, 3, 1)           # [B, 133, 133, C]
    zz = np.empty((B, H4, W4, 4 * C), np.float32)
    zz[..., 0 * C:1 * C] = xcl0[:, :H4, :W4, :]
    zz[..., 1 * C:2 * C] = xcl0[:, :H4, 1:W4 + 1, :]
    zz[..., 2 * C:3 * C] = xcl0[:, 1:H4 + 1, :W4, :]
    zz[..., 3 * C:4 * C] = xcl0[:, 1:H4 + 1, 1:W4 + 1, :]
    xcl_full = zz

    # offset conv weights: [c, kk*18], lhsT per tap
    wofft = np.ascontiguousarray(
        offset_w.reshape(OC, C, KK).transpose(1, 2, 0)).reshape(C, KK * OC)
    woffb = offset_b.reshape(OC, 1)
    # main conv weights: [128, 5*64]; block j rows (t2*64+c), cols o
    wr = weight.reshape(O, C, KK)
    wmain = np.zeros((128, 5 * O), np.float32)
    for j in range(5):
        for t2 in range(2):
            kk = 2 * j + t2
            if kk >= KK:
                break
            wmain[t2 * C:(t2 + 1) * C, j * O:(j + 1) * O] = wr[:, :, kk].T
    wb = bias.reshape(O, 1)
    identm = np.eye(128, dtype=np.float32)

    p = np.arange(128, dtype=np.float32)
    g = np.arange(NG, dtype=np.float32)
    kki = (np.arange(KK) // K).astype(np.float32)
    kkj = (np.arange(KK) % K).astype(np.float32)
    # basex[p, g, kk] = p - 1 + kj
    basex = (p[:, None, None] - 1.0 + kkj[None, None, :]) \
        + 0.0 * g[None, :, None]
    basex = np.ascontiguousarray(
        np.broadcast_to(basex, (128, NG, KK)), np.float32).reshape(128, NG * KK)

    in_maps = []
    for core in range(8):
        b, h0 = core // 2, (core % 2) * HL
        by = np.broadcast_to(
            (h0 + g)[None, :, None] - 1.0 + kki[None, None, :],
            (128, NG, KK))
        in_maps.append({
            "xp": np.ascontiguousarray(
                xp_full[b, :, h0:h0 + H2, :]).reshape(C, H2 * W2),
            "xcl": np.ascontiguousarray(xcl_full[b]).reshape(H4 * W4, 4 * C),
            "wofft": wofft, "woffb": woffb,
            "wmain": wmain, "wb": wb,
            "basey": np.ascontiguousarray(by, np.float32).reshape(128, NG * KK),
            "basex": basex,
            "ident": identm,
        })
    return in_maps


def kernel(x, weight, bias, offset_w, offset_b):
    nc = _get_nc()
    in_maps = make_core_inputs(x, weight, bias, offset_w, offset_b)
    res = run_bass_kernel_spmd(nc, in_maps, list(range(8)))
    out_full = np.empty((B, O, H, W), np.float32)
    for core in range(8):
        b, h0 = core // 2, (core % 2) * HL
        out_full[b, :, h0:h0 + HL, :] = res.results[core]["out"].reshape(O, HL, W)
    return out_full

